# revision 28
# baseline (speedup 1.0000x reference)
"""KMeans assignment (vq_codebook) Trainium2 kernel.

argmin_k ||x_b - c_k||^2 for X[65536,1024], C[1024,1024], 8 NeuronCores,
data-parallel over the batch (8192 rows/core), centroids replicated.

Math: argmin_k d2 = argmax_k (X@C^T - ||c||^2/2); row term ||x||^2 dropped.
The cross term is a single fp16 matmul pass (one row/cycle PE rate; fp16
input rounding costs ~36/65536 near-tie argmax flips on hardware, inside
the rel-err budget, and halves all input DMA bytes so the PE never waits
on data). The ||c||^2/2 bias is precomputed on the host and subtracted on
the Vector engine.

Argmax per tile: DVE folds the 1024 scores to 512 (tensor_max), then
max/max_index scan only the folded half — keeping DVE at ~2.9us/tile,
under the PE's 3.4us/tile, so the Tensor engine paces the kernel at
~97% occupancy. The which-half bit needed to decode the folded index is
computed on the otherwise idle Pool engine as a (lo - hi) difference row
(Pool supports subtract but not compare opcodes, and cannot touch PSUM)
and shipped to the host, which decodes the true index by sign. Warmup
matmuls on a tiny zeroed tile eat the PE p-state ramp while the first
DMAs land.
"""
import numpy as np
import concourse.bacc as bacc
import concourse.mybir as mybir
from concourse.tile import TileContext
from concourse.bass_utils import run_bass_kernel_spmd

B, F, K = 65536, 1024, 1024
NCORES = 8
BL = B // NCORES          # rows per core
P = 128
FCH = F // P              # 8 contraction chunks
NH = 512                  # psum half (max fp32 moving operand / bank)
# X DMA block sizes (columns). With fp16 inputs the transfers outrun the
# PE comfortably, so uniform blocks suffice (a ramp no longer helps).
BLOCKS = [1024] * 8
NWARM = 24                # p-state warmup matmuls
DT = mybir.dt.float16

_NC_CACHE = {}


def _build(bl):
    assert sum(BLOCKS) == bl
    nb = bl // P
    nc = bacc.Bacc("TRN2", target_bir_lowering=False)
    x = nc.dram_tensor("x", [F, bl], DT, kind="ExternalInput")
    c = nc.dram_tensor("c", [F, K], mybir.dt.float16,
                        kind="ExternalInput")
    c2h = nc.dram_tensor("c2h", [K], mybir.dt.float32, kind="ExternalInput")
    out = nc.dram_tensor("out", [nb, P, 1], mybir.dt.uint32, kind="ExternalOutput")
    om = nc.dram_tensor("om", [nb, P, NH], mybir.dt.float32,
                        kind="ExternalOutput")

    with TileContext(nc) as tc:
        with (
            tc.tile_pool(name="cres", bufs=1) as cres,
            tc.tile_pool(name="xp", bufs=2) as xp,
            tc.tile_pool(name="work", bufs=3) as work,
            tc.tile_pool(name="psp", bufs=4, space="PSUM") as psp,
        ):
            # Warmup: dummy matmuls into a discarded psum tile eat the PE
            # low/mid p-state ramp while the first input DMAs are in flight,
            # so real matmuls start at full clock.
            dumt = work.tile([P, P], DT, tag="warm")
            nc.vector.memzero(dumt)
            dps = psp.tile([P, K], mybir.dt.float32, tag="ps")
            for w in range(NWARM):
                nc.tensor.matmul(dps[:, 0:P], dumt, dumt,
                                 start=(w == 0), stop=(w == NWARM - 1))

            def load_chunk(c0, ncols, f):
                t = xp.tile([P, ncols], DT, tag=f"x{f}", name=f"xt{f}")
                nc.sync.dma_start(t, x[f * P:(f + 1) * P, c0:c0 + ncols])
                return t

            # Interleave resident-C chunk loads with block-0 X chunk loads so
            # the f-th matmul of tile 0 only waits on the f-th pair; the c2
            # broadcast (needed by the first sub, several tiles later) rides
            # along early.
            c_sb = []
            blk0 = []
            c2b = cres.tile([P, K], mybir.dt.float32)
            for f in range(FCH):
                t_c = cres.tile([P, K], mybir.dt.float16, tag=f"c{f}")
                nc.sync.dma_start(t_c, c[f * P:(f + 1) * P, :])
                c_sb.append(t_c)
                blk0.append(load_chunk(0, BLOCKS[0], f))
            # c2b rides the serial DMA stream AFTER block 0 — it's not needed
            # until the first sub (~20us), and placing it earlier delays the
            # tile-0 inputs that gate the whole pipeline.
            nc.sync.dma_start(c2b, c2h[None, :].to_broadcast([P, K]))

            t = 0
            c0 = 0
            for bi, ncols in enumerate(BLOCKS):
                x_t = blk0 if bi == 0 else [load_chunk(c0, ncols, f)
                                            for f in range(FCH)]
                for i in range(ncols // P):
                    ps = psp.tile([P, K], mybir.dt.float32, tag="ps")
                    for f in range(FCH):
                        first = f == 0
                        last = f == FCH - 1
                        w = x_t[f][:, i * P:(i + 1) * P]
                        nc.tensor.matmul(ps[:, 0:NH], w, c_sb[f][:, 0:NH],
                                         start=first, stop=last)
                        nc.tensor.matmul(ps[:, NH:K], w, c_sb[f][:, NH:K],
                                         start=first, stop=last)
                    a_sb = work.tile([P, K], mybir.dt.float32, tag="a")
                    nc.vector.tensor_sub(a_sb, ps, c2b)
                    # DVE folds the 1024 scores to 512 so max/max_index scan
                    # half the data (DVE 2.97us/tile, under the PE's 3.41 —
                    # the PE becomes the pacer). The which-half bit needed to
                    # decode the folded index is computed on the otherwise
                    # idle Pool engine and shipped to the host, so nothing
                    # feeds back into the DVE chain.
                    m = work.tile([P, NH], mybir.dt.float32, tag="m")
                    nc.vector.tensor_max(m, a_sb[:, 0:NH], a_sb[:, NH:K])
                    d = work.tile([P, NH], mybir.dt.float32, tag="d")
                    nc.gpsimd.tensor_sub(d, a_sb[:, 0:NH], a_sb[:, NH:K])
                    mx = work.tile([P, 8], mybir.dt.float32, tag="mx")
                    nc.vector.max(out=mx, in_=m)
                    ix = work.tile([P, 8], mybir.dt.uint32, tag="ix")
                    nc.vector.max_index(ix, mx, m)
                    nc.sync.dma_start(out[t], ix[:, 0:1])
                    nc.scalar.dma_start(om[t], d)
                    t += 1
                c0 += ncols
    nc.finalize()
    return nc


def _get_nc(bl):
    if bl not in _NC_CACHE:
        _NC_CACHE[bl] = _build(bl)
    return _NC_CACHE[bl]


def _prep_in_maps(X, C):
    X = np.ascontiguousarray(np.asarray(X, dtype=np.float32))
    C = np.ascontiguousarray(np.asarray(C, dtype=np.float32))
    assert X.shape == (B, F) and C.shape == (K, F)
    xt = np.ascontiguousarray(X.T.astype(np.float16))
    ct = np.ascontiguousarray(C.T.astype(np.float16))
    c2h = (0.5 * np.sum(C.astype(np.float64) ** 2, axis=1)).astype(np.float32)
    in_maps = []
    for cid in range(NCORES):
        sl = slice(cid * BL, (cid + 1) * BL)
        in_maps.append({
            "x": np.ascontiguousarray(xt[:, sl]),
            "c": ct,
            "c2h": c2h,
        })
    return in_maps


def kernel(X, centroids):
    nc = _get_nc(BL)
    in_maps = _prep_in_maps(X, centroids)
    res = run_bass_kernel_spmd(nc, in_maps, core_ids=list(range(NCORES)))
    outs = []
    for r in res.results:
        j = r["out"].reshape(-1).astype(np.int64)          # folded index
        dm = r["om"].reshape(-1, NH)                       # a_lo - a_hi rows
        d = np.take_along_axis(dm, j[:, None], axis=1).reshape(-1)
        outs.append(np.where(d >= 0, j, j + NH))           # decode true k
    return np.concatenate(outs).astype(np.int32)
